# revision 1
# baseline (speedup 1.0000x reference)
"""Trainium2 Bass kernel for nn_Decoder (dense MLP).

Computes out = relu(V @ W1 + b1) @ W2 + b2 for V [262144, 1024],
W1 [1024, 128], W2 [128, 4].

Strategy
--------
Data-parallel over 8 NeuronCores: V is sharded along rows (32768 rows per
core); the small weights are replicated. Each core's V shard is transposed
on the host to [1024, 32768] so the contraction dim (1024) lands on SBUF
partitions with fully contiguous DMA loads — no on-chip transposes.

Per core, the kernel computes h.T = W1.T @ V.T via PSUM-accumulated
matmuls over 8 K-chunks (lhsT = the natural W1 layout), applies
ReLU(+b1) on the scalar engine reading PSUM, then out.T = W2.T @ h.T on
the tensor engine, adds b2 on the vector engine, and stores out.T
[4, 32768] contiguously. The host transposes the gathered outputs back.

Precision modes (KERNEL_MODE env var):
  f32    — plain fp32 matmuls (4 cycles/row on PE).
  bf16   — single-pass bf16 (half the DMA bytes, ~2e-3 rel err).
  f16    — single-pass fp16 (half the DMA bytes, ~3e-4 rel err).
  bf16x2 — hi/lo bf16 split, 3 matmul passes (~5e-6 rel err).
  f16x2  — hi/lo fp16 split, 3 matmul passes (~1e-6, fp32-grade; default).

Measured on HW (8 cores, full size): f16x2 ≈ 460 us, rel err 1.1e-6.
"""

import os
import sys

import numpy as np

for _p in ("/opt/trn_rl_repo", "/root/.axon_site/_ro/trn_rl_repo"):
    if os.path.isdir(_p) and _p not in sys.path:
        sys.path.insert(0, _p)

import concourse.bass as bass
import concourse.mybir as mybir
import concourse.tile as tile
from concourse import bacc
from concourse.bass_utils import run_bass_kernel_spmd

NCORES = 8
NN = 262144
IN_DIM = 1024
HIDDEN = 128
OUT_DIM = 4
R = NN // NCORES  # rows per core

P = 128           # SBUF partitions
KC = IN_DIM // P  # 8 k-chunks
CHUNK = 512       # rows per PSUM accumulation tile (one PSUM bank)
GROUP = 2048      # rows per DMA group
DATA_BUFS = 2     # prefetch depth for V-group tiles

MODE = os.environ.get("KERNEL_MODE", "f16x2")

_TWO_PASS = {"bf16x2", "f16x2"}
_last_results = None  # exposed for test harness (exec_time_ns etc.)


def _moving_dtype(mode):
    if mode in ("bf16", "bf16x2"):
        return mybir.dt.bfloat16
    if mode in ("f16", "f16x2"):
        return mybir.dt.float16
    return mybir.dt.float32


def build_nc(mode=MODE, rows=R):
    """Build the SPMD Bass program for one core."""
    f32 = mybir.dt.float32
    mdt = _moving_dtype(mode)
    two_pass = mode in _TWO_PASS

    nc = bacc.Bacc("TRN2")

    vth_d = nc.declare_dram_parameter("VTH", [IN_DIM, rows], mdt, isOutput=False)
    if two_pass:
        vtl_d = nc.declare_dram_parameter("VTL", [IN_DIM, rows], mdt, isOutput=False)
    w1h_d = nc.declare_dram_parameter("W1H", [IN_DIM, HIDDEN], mdt, isOutput=False)
    if two_pass:
        w1l_d = nc.declare_dram_parameter("W1L", [IN_DIM, HIDDEN], mdt, isOutput=False)
    b1_d = nc.declare_dram_parameter("B1", [HIDDEN, 1], f32, isOutput=False)
    if two_pass:
        w2h_d = nc.declare_dram_parameter("W2H", [HIDDEN, OUT_DIM], mdt, isOutput=False)
        w2l_d = nc.declare_dram_parameter("W2L", [HIDDEN, OUT_DIM], mdt, isOutput=False)
    else:
        w2_d = nc.declare_dram_parameter("W2", [HIDDEN, OUT_DIM], f32, isOutput=False)
    b2_d = nc.declare_dram_parameter("B2", [OUT_DIM, 1], f32, isOutput=False)
    out_d = nc.declare_dram_parameter("OUT", [OUT_DIM, rows], f32, isOutput=True)

    ngroups = rows // GROUP
    nchunk = GROUP // CHUNK

    with tile.TileContext(nc) as tc:
        with (
            tc.tile_pool(name="const", bufs=1) as cpool,
            tc.tile_pool(name="data", bufs=DATA_BUFS) as dpool,
            tc.tile_pool(name="work", bufs=3) as wpool,
            tc.tile_pool(name="psum1", bufs=4, space="PSUM") as ppool,
            tc.tile_pool(name="psum2", bufs=2, space="PSUM") as opool,
        ):
            # --- constants (loaded once) ---
            w1h_sb = cpool.tile([P, KC, HIDDEN], mdt)
            nc.sync.dma_start(
                w1h_sb[:], w1h_d[:].rearrange("(c p) h -> p c h", p=P)
            )
            if two_pass:
                w1l_sb = cpool.tile([P, KC, HIDDEN], mdt)
                nc.sync.dma_start(
                    w1l_sb[:], w1l_d[:].rearrange("(c p) h -> p c h", p=P)
                )
            b1_sb = cpool.tile([HIDDEN, 1], f32)
            nc.sync.dma_start(b1_sb[:], b1_d[:])
            if two_pass:
                w2h_sb = cpool.tile([HIDDEN, OUT_DIM], mdt)
                nc.sync.dma_start(w2h_sb[:], w2h_d[:])
                w2l_sb = cpool.tile([HIDDEN, OUT_DIM], mdt)
                nc.sync.dma_start(w2l_sb[:], w2l_d[:])
            else:
                w2_sb = cpool.tile([HIDDEN, OUT_DIM], f32)
                nc.sync.dma_start(w2_sb[:], w2_d[:])
            b2_sb = cpool.tile([OUT_DIM, 1], f32)
            nc.sync.dma_start(b2_sb[:], b2_d[:])

            vth_view = vth_d[:].rearrange("(c p) (g n) -> g p c n", p=P, n=GROUP)
            if two_pass:
                vtl_view = vtl_d[:].rearrange("(c p) (g n) -> g p c n", p=P, n=GROUP)
            out_view = out_d[:].rearrange("o (m n) -> m o n", n=CHUNK)

            for g in range(ngroups):
                vth = dpool.tile([P, KC, GROUP], mdt, tag="vth")
                vtl = None
                if two_pass:
                    vtl = dpool.tile([P, KC, GROUP], mdt, tag="vtl")
                if g == 0:
                    # split the first group per-chunk (hi/lo interleaved)
                    # so PE starts as early as possible
                    for u in range(nchunk):
                        slv = slice(u * CHUNK, (u + 1) * CHUNK)
                        nc.sync.dma_start(vth[:, :, slv], vth_view[g][:, :, slv])
                        if two_pass:
                            nc.sync.dma_start(vtl[:, :, slv], vtl_view[g][:, :, slv])
                else:
                    nc.sync.dma_start(vth[:], vth_view[g])
                    if two_pass:
                        nc.sync.dma_start(vtl[:], vtl_view[g])

                for u in range(nchunk):
                    sl = slice(u * CHUNK, (u + 1) * CHUNK)
                    mms = []
                    for c in range(KC):
                        mms.append((w1h_sb[:, c, :], vth[:, c, sl]))
                        if two_pass:
                            mms.append((w1h_sb[:, c, :], vtl[:, c, sl]))
                            mms.append((w1l_sb[:, c, :], vth[:, c, sl]))

                    ps = ppool.tile([HIDDEN, CHUNK], f32, tag="ps")
                    n_mm = len(mms)
                    for i, (wap, vap) in enumerate(mms):
                        nc.tensor.matmul(
                            ps[:], wap, vap,
                            start=(i == 0), stop=(i == n_mm - 1),
                        )

                    po = opool.tile([OUT_DIM, CHUNK], f32, tag="po")
                    if two_pass:
                        # Split h into hi/lo halves so layer 2 also runs at
                        # 1 cycle/row while staying fp32-grade:
                        #   hh = mdt(relu(ps+b1));  hl = mdt(relu_f32 - hh)
                        hh = wpool.tile([HIDDEN, CHUNK], mdt, tag="hh")
                        nc.scalar.activation(
                            hh[:], ps[:],
                            mybir.ActivationFunctionType.Relu,
                            bias=b1_sb[:],
                        )
                        hf = wpool.tile([HIDDEN, CHUNK], f32, tag="hf")
                        nc.vector.tensor_scalar(
                            hf[:], ps[:], b1_sb[:], 0.0,
                            op0=mybir.AluOpType.add,
                            op1=mybir.AluOpType.max,
                        )
                        hl = wpool.tile([HIDDEN, CHUNK], mdt, tag="hl")
                        nc.vector.tensor_sub(hl[:], hf[:], hh[:])
                        nc.tensor.matmul(po[:], w2h_sb[:], hh[:], start=True, stop=False)
                        nc.tensor.matmul(po[:], w2h_sb[:], hl[:], start=False, stop=False)
                        nc.tensor.matmul(po[:], w2l_sb[:], hh[:], start=False, stop=True)
                    else:
                        h_sb = wpool.tile([HIDDEN, CHUNK], f32, tag="h")
                        nc.scalar.activation(
                            h_sb[:], ps[:],
                            mybir.ActivationFunctionType.Relu,
                            bias=b1_sb[:],
                        )
                        w2ap, hap = w2_sb[:], h_sb[:]
                        nc.tensor.matmul(po[:], w2ap, hap, start=True, stop=True)

                    o_sb = wpool.tile([OUT_DIM, CHUNK], f32, tag="o")
                    nc.vector.tensor_scalar_add(o_sb[:], po[:], b2_sb[:])

                    nc.scalar.dma_start(out_view[g * nchunk + u], o_sb[:])

    return nc


def _split_hi_lo(x, np_dt):
    hi = x.astype(np_dt)
    lo = (x - hi.astype(np.float32)).astype(np_dt)
    return hi, lo


def kernel(V, W1, b1, W2, b2):
    global _last_results
    mode = MODE
    mdt = _moving_dtype(mode)
    np_dt = {
        mybir.dt.float32: np.float32,
        mybir.dt.bfloat16: None,  # filled below (ml_dtypes)
        mybir.dt.float16: np.float16,
    }[mdt]
    if np_dt is None:
        import ml_dtypes

        np_dt = ml_dtypes.bfloat16
    two_pass = mode in _TWO_PASS

    V = np.asarray(V, dtype=np.float32)
    W1 = np.asarray(W1, dtype=np.float32)
    b1 = np.asarray(b1, dtype=np.float32)
    W2 = np.asarray(W2, dtype=np.float32)
    b2 = np.asarray(b2, dtype=np.float32)

    common = {
        "B1": np.ascontiguousarray(b1.reshape(HIDDEN, 1)),
        "B2": np.ascontiguousarray(b2.reshape(OUT_DIM, 1)),
    }
    if two_pass:
        common["W1H"], common["W1L"] = _split_hi_lo(W1, np_dt)
        common["W2H"], common["W2L"] = _split_hi_lo(W2, np_dt)
    else:
        common["W1H"] = W1.astype(np_dt)
        common["W2"] = np.ascontiguousarray(W2)

    in_maps = []
    for c in range(NCORES):
        shard = V[c * R : (c + 1) * R]  # [R, IN_DIM]
        if two_pass:
            hi, lo = _split_hi_lo(shard, np_dt)
            m = {
                "VTH": np.ascontiguousarray(hi.T),
                "VTL": np.ascontiguousarray(lo.T),
            }
        else:
            m = {"VTH": np.ascontiguousarray(shard.T.astype(np_dt))}
        m.update(common)
        in_maps.append(m)

    nc = build_nc(mode, R)
    nc.finalize()
    res = run_bass_kernel_spmd(nc, in_maps, list(range(NCORES)))
    _last_results = res

    out = np.concatenate(
        [np.asarray(r["OUT"]).T for r in res.results], axis=0
    ).astype(np.float32)
    return out



# revision 2
# speedup vs baseline: 1.1167x; 1.1167x over previous
"""Trainium2 Bass kernel for nn_Decoder (dense MLP).

Computes out = relu(V @ W1 + b1) @ W2 + b2 for V [262144, 1024],
W1 [1024, 128], W2 [128, 4].

Strategy
--------
Data-parallel over 8 NeuronCores: V is sharded along rows (32768 rows per
core); the small weights are replicated. Each core's V shard is transposed
on the host to [1024, 32768] so the contraction dim (1024) lands on SBUF
partitions with fully contiguous DMA loads — no on-chip transposes.

Per core, the kernel computes h.T = W1.T @ V.T via PSUM-accumulated
matmuls over 8 K-chunks (lhsT = the natural W1 layout), applies
ReLU(+b1) on the scalar engine reading PSUM (emitting f16 h), then
out.T = W2.T @ h.T as a single f16 matmul, adds b2 on the vector
engine, and stores out.T [4, 32768] contiguously. The host transposes
the gathered outputs back.

The kernel is HBM-bound: V in f16 is 64 MiB/core against the ~358 GB/s
per-core HBM ceiling (~187 us). Everything else is engineered to keep
the 16 SDMA engines continuously busy: the first V chunk's DMA is
issued before anything else on the sync ring, weights load on the
scalar ring in parallel, and 4 group buffers of prefetch depth keep
the DMA queues fed. All matmuls run in f16 (1 col/cycle @ 2.4 GHz
warm) so the PE (~140 us) stays off the critical path.

Precision modes (KERNEL_MODE env var):
  f32    — plain fp32 matmuls (4x PE cycles, 2x DMA bytes).
  bf16   — single-pass bf16 (~3e-3 rel err).
  f16    — single-pass fp16 (~4e-4 rel err; default).
"""

import os
import sys

import numpy as np

for _p in ("/opt/trn_rl_repo", "/root/.axon_site/_ro/trn_rl_repo"):
    if os.path.isdir(_p) and _p not in sys.path:
        sys.path.insert(0, _p)

import concourse.bass as bass
import concourse.mybir as mybir
import concourse.tile as tile
from concourse import bacc
from concourse.bass_utils import run_bass_kernel_spmd

NCORES = 8
NN = 262144
IN_DIM = 1024
HIDDEN = 128
OUT_DIM = 4
R = NN // NCORES  # rows per core

P = 128           # SBUF partitions
KC = IN_DIM // P  # 8 k-chunks
CHUNK = 512       # rows per PSUM accumulation tile (one PSUM bank)
GROUP = 2048      # rows per DMA group
DATA_BUFS = 4     # prefetch depth for V-group tiles

MODE = os.environ.get("KERNEL_MODE", "f16")

_last_results = None  # exposed for test harness (exec_time_ns etc.)


def _moving_dtype(mode):
    if mode == "bf16":
        return mybir.dt.bfloat16
    if mode == "f16":
        return mybir.dt.float16
    return mybir.dt.float32


def build_nc(mode=MODE, rows=R):
    """Build the SPMD Bass program for one core."""
    f32 = mybir.dt.float32
    mdt = _moving_dtype(mode)

    nc = bacc.Bacc("TRN2")

    vth_d = nc.declare_dram_parameter("VTH", [IN_DIM, rows], mdt, isOutput=False)
    w1_d = nc.declare_dram_parameter("W1H", [IN_DIM, HIDDEN], mdt, isOutput=False)
    b1_d = nc.declare_dram_parameter("B1", [HIDDEN, 1], f32, isOutput=False)
    w2_d = nc.declare_dram_parameter("W2F", [HIDDEN, OUT_DIM], mdt, isOutput=False)
    b2_d = nc.declare_dram_parameter("B2", [OUT_DIM, 1], f32, isOutput=False)
    out_d = nc.declare_dram_parameter("OUT", [OUT_DIM, rows], f32, isOutput=True)

    ngroups = rows // GROUP
    nchunk = GROUP // CHUNK

    with tile.TileContext(nc) as tc:
        with (
            tc.tile_pool(name="const", bufs=1) as cpool,
            tc.tile_pool(name="data", bufs=DATA_BUFS) as dpool,
            tc.tile_pool(name="work", bufs=3) as wpool,
            tc.tile_pool(name="psum1", bufs=4, space="PSUM") as ppool,
            tc.tile_pool(name="psum2", bufs=2, space="PSUM") as opool,
        ):
            vth_view = vth_d[:].rearrange("(c p) (g n) -> g p c n", p=P, n=GROUP)
            out_view = out_d[:].rearrange("o (m n) -> m o n", n=CHUNK)

            # Bootstrap: put the first V chunk's DMA at the head of the
            # sync ring so the HBM stream starts immediately; weights
            # load on the scalar ring in parallel.
            vth0 = dpool.tile([P, KC, GROUP], mdt, tag="vth")
            nc.sync.dma_start(vth0[:, :, 0:CHUNK], vth_view[0][:, :, 0:CHUNK])

            w1_sb = cpool.tile([P, KC, HIDDEN], mdt)
            nc.scalar.dma_start(
                w1_sb[:], w1_d[:].rearrange("(c p) h -> p c h", p=P)
            )
            b1_sb = cpool.tile([HIDDEN, 1], f32)
            nc.scalar.dma_start(b1_sb[:], b1_d[:])
            w2_sb = cpool.tile([HIDDEN, OUT_DIM], mdt)
            nc.scalar.dma_start(w2_sb[:], w2_d[:])
            b2_sb = cpool.tile([OUT_DIM, 1], f32)
            nc.scalar.dma_start(b2_sb[:], b2_d[:])

            nc.sync.dma_start(vth0[:, :, CHUNK:2 * CHUNK],
                              vth_view[0][:, :, CHUNK:2 * CHUNK])
            nc.sync.dma_start(vth0[:, :, 2 * CHUNK:], vth_view[0][:, :, 2 * CHUNK:])

            for g in range(ngroups):
                if g == 0:
                    vth = vth0
                else:
                    vth = dpool.tile([P, KC, GROUP], mdt, tag="vth")
                    nc.sync.dma_start(vth[:], vth_view[g])

                for u in range(nchunk):
                    sl = slice(u * CHUNK, (u + 1) * CHUNK)

                    ps = ppool.tile([HIDDEN, CHUNK], f32, tag="ps")
                    for c in range(KC):
                        nc.tensor.matmul(
                            ps[:], w1_sb[:, c, :], vth[:, c, sl],
                            start=(c == 0), stop=(c == KC - 1),
                        )

                    h_sb = wpool.tile([HIDDEN, CHUNK], mdt, tag="h")
                    nc.scalar.activation(
                        h_sb[:], ps[:],
                        mybir.ActivationFunctionType.Relu,
                        bias=b1_sb[:],
                    )

                    po = opool.tile([OUT_DIM, CHUNK], f32, tag="po")
                    nc.tensor.matmul(po[:], w2_sb[:], h_sb[:], start=True, stop=True)

                    o_sb = wpool.tile([OUT_DIM, CHUNK], f32, tag="o")
                    nc.vector.tensor_scalar_add(o_sb[:], po[:], b2_sb[:])

                    nc.scalar.dma_start(out_view[g * nchunk + u], o_sb[:])

    return nc


def kernel(V, W1, b1, W2, b2):
    global _last_results
    mode = MODE
    mdt = _moving_dtype(mode)
    np_dt = {
        mybir.dt.float32: np.float32,
        mybir.dt.bfloat16: None,  # filled below (ml_dtypes)
        mybir.dt.float16: np.float16,
    }[mdt]
    if np_dt is None:
        import ml_dtypes

        np_dt = ml_dtypes.bfloat16

    V = np.asarray(V, dtype=np.float32)
    W1 = np.asarray(W1, dtype=np.float32)
    b1 = np.asarray(b1, dtype=np.float32)
    W2 = np.asarray(W2, dtype=np.float32)
    b2 = np.asarray(b2, dtype=np.float32)

    common = {
        "W1H": np.ascontiguousarray(W1.astype(np_dt)),
        "B1": np.ascontiguousarray(b1.reshape(HIDDEN, 1)),
        "W2F": np.ascontiguousarray(W2.astype(np_dt)),
        "B2": np.ascontiguousarray(b2.reshape(OUT_DIM, 1)),
    }

    in_maps = []
    for c in range(NCORES):
        shard = V[c * R : (c + 1) * R]  # [R, IN_DIM]
        m = {"VTH": np.ascontiguousarray(shard.T.astype(np_dt))}
        m.update(common)
        in_maps.append(m)

    nc = build_nc(mode, R)
    nc.finalize()
    res = run_bass_kernel_spmd(nc, in_maps, list(range(NCORES)))
    _last_results = res

    out = np.concatenate(
        [np.asarray(r["OUT"]).T for r in res.results], axis=0
    ).astype(np.float32)
    return out


# revision 7
# speedup vs baseline: 1.2238x; 1.0959x over previous
"""Trainium2 Bass kernel for nn_Decoder (dense MLP).

Computes out = relu(V @ W1 + b1) @ W2 + b2 for V [262144, 1024],
W1 [1024, 128], W2 [128, 4].

Strategy
--------
Data-parallel over 8 NeuronCores: V is sharded along rows (32768 rows per
core); the small weights are replicated. Each core's V shard is transposed
on the host to [1024, 32768] so the contraction dim (1024) lands on SBUF
partitions with fully contiguous DMA loads — no on-chip transposes.

Per core, the kernel computes h.T = W1.T @ V.T via PSUM-accumulated
matmuls over 8 K-chunks (lhsT = the natural W1 layout), applies
ReLU(+b1) on the scalar engine reading PSUM (emitting f16 h), then
out.T = W2.T @ h.T as a single f16 matmul, adds b2 on the vector
engine, and stores out.T [4, 32768] contiguously. The host transposes
the gathered outputs back.

The kernel is HBM-bound: V in f16 is 64 MiB/core against the ~358 GB/s
per-core HBM ceiling (~187 us). Everything else is engineered to keep
the 16 SDMA engines continuously busy: the first V chunk's DMA is
issued before anything else on the sync ring, weights load on the
scalar ring in parallel, and 4 group buffers of prefetch depth keep
the DMA queues fed. All matmuls run in f16 (1 col/cycle @ 2.4 GHz
warm) so the PE (~140 us) stays off the critical path.

Precision modes (KERNEL_MODE env var):
  f32    — plain fp32 matmuls (4x PE cycles, 2x DMA bytes).
  bf16   — single-pass bf16 (~3e-3 rel err).
  f16    — single-pass fp16 (~4e-4 rel err; default).
"""

import os
import sys

import numpy as np

for _p in ("/opt/trn_rl_repo", "/root/.axon_site/_ro/trn_rl_repo"):
    if os.path.isdir(_p) and _p not in sys.path:
        sys.path.insert(0, _p)

import concourse.bass as bass
import concourse.mybir as mybir
import concourse.tile as tile
from concourse import bacc
from concourse.bass_utils import run_bass_kernel_spmd

NCORES = 8
NN = 262144
IN_DIM = 1024
HIDDEN = 128
OUT_DIM = 4
R = NN // NCORES  # rows per core

P = 128           # SBUF partitions
KC = IN_DIM // P  # 8 k-chunks
CHUNK = 512       # rows per PSUM accumulation tile (one PSUM bank)
GROUP = 2048      # rows per DMA group
DATA_BUFS = 6     # prefetch depth for V-group tiles

MODE = os.environ.get("KERNEL_MODE", "f16")

_last_results = None  # exposed for test harness (exec_time_ns etc.)


def _moving_dtype(mode):
    if mode == "bf16":
        return mybir.dt.bfloat16
    if mode == "f16":
        return mybir.dt.float16
    return mybir.dt.float32


def build_nc(mode=MODE, rows=R):
    """Build the SPMD Bass program for one core."""
    f32 = mybir.dt.float32
    mdt = _moving_dtype(mode)

    nc = bacc.Bacc("TRN2")

    vth_d = nc.declare_dram_parameter("VTH", [IN_DIM, rows], mdt, isOutput=False)
    # W1 arrives host-prepacked in SBUF layout [P, KC*HIDDEN] so its DMA
    # moves 2 KB contiguous lines (128 descriptors) instead of 256 B ones.
    w1_d = nc.declare_dram_parameter("W1P", [P, KC * HIDDEN], mdt, isOutput=False)
    b1_d = nc.declare_dram_parameter("B1", [HIDDEN, 1], f32, isOutput=False)
    w2_d = nc.declare_dram_parameter("W2F", [HIDDEN, OUT_DIM], mdt, isOutput=False)
    b2_d = nc.declare_dram_parameter("B2", [OUT_DIM, 1], f32, isOutput=False)
    out_d = nc.declare_dram_parameter("OUT", [OUT_DIM, rows], f32, isOutput=True)

    ngroups = rows // GROUP
    nchunk = GROUP // CHUNK

    with tile.TileContext(nc) as tc:
        with (
            tc.tile_pool(name="const", bufs=1) as cpool,
            tc.tile_pool(name="data", bufs=DATA_BUFS) as dpool,
            tc.tile_pool(name="work", bufs=3) as wpool,
            tc.tile_pool(name="psum1", bufs=4, space="PSUM") as ppool,
            tc.tile_pool(name="psum2", bufs=2, space="PSUM") as opool,
        ):
            vth_view = vth_d[:].rearrange("(c p) (g n) -> g p c n", p=P, n=GROUP)
            out_view = out_d[:].rearrange("o (m n) -> m o n", n=CHUNK)

            # Bootstrap: put the first V chunk's DMA at the head of the
            # sync ring so the HBM stream starts immediately; weights
            # load on the scalar ring in parallel.
            vth0 = dpool.tile([P, KC, GROUP], mdt, tag="vth")
            nc.sync.dma_start(vth0[:, :, 0:CHUNK], vth_view[0][:, :, 0:CHUNK])

            w1_sb = cpool.tile([P, KC, HIDDEN], mdt)
            nc.scalar.dma_start(
                w1_sb[:], w1_d[:].rearrange("p (c h) -> p c h", c=KC)
            )
            b1_sb = cpool.tile([HIDDEN, 1], f32)
            nc.scalar.dma_start(b1_sb[:], b1_d[:])
            w2_sb = cpool.tile([HIDDEN, OUT_DIM], mdt)
            nc.scalar.dma_start(w2_sb[:], w2_d[:])
            b2_sb = cpool.tile([OUT_DIM, 1], f32)
            nc.scalar.dma_start(b2_sb[:], b2_d[:])

            nc.sync.dma_start(vth0[:, :, CHUNK:2 * CHUNK],
                              vth_view[0][:, :, CHUNK:2 * CHUNK])
            nc.sync.dma_start(vth0[:, :, 2 * CHUNK:], vth_view[0][:, :, 2 * CHUNK:])

            for g in range(ngroups):
                if g == 0:
                    vth = vth0
                else:
                    vth = dpool.tile([P, KC, GROUP], mdt, tag="vth")
                    if g == ngroups - 1:
                        # Split the last group per chunk so the trailing
                        # compute overlaps the tail of the DMA stream.
                        for u in range(nchunk):
                            slu = slice(u * CHUNK, (u + 1) * CHUNK)
                            nc.sync.dma_start(vth[:, :, slu], vth_view[g][:, :, slu])
                    else:
                        nc.sync.dma_start(vth[:], vth_view[g])

                for u in range(nchunk):
                    sl = slice(u * CHUNK, (u + 1) * CHUNK)

                    ps = ppool.tile([HIDDEN, CHUNK], f32, tag="ps")
                    for c in range(KC):
                        nc.tensor.matmul(
                            ps[:], w1_sb[:, c, :], vth[:, c, sl],
                            start=(c == 0), stop=(c == KC - 1),
                        )

                    h_sb = wpool.tile([HIDDEN, CHUNK], mdt, tag="h")
                    nc.scalar.activation(
                        h_sb[:], ps[:],
                        mybir.ActivationFunctionType.Relu,
                        bias=b1_sb[:],
                    )

                    po = opool.tile([OUT_DIM, CHUNK], f32, tag="po")
                    nc.tensor.matmul(po[:], w2_sb[:], h_sb[:], start=True, stop=True)

                    o_sb = wpool.tile([OUT_DIM, CHUNK], f32, tag="o")
                    nc.vector.tensor_scalar_add(o_sb[:], po[:], b2_sb[:])

                    nc.scalar.dma_start(out_view[g * nchunk + u], o_sb[:])

    return nc


def kernel(V, W1, b1, W2, b2):
    global _last_results
    mode = MODE
    mdt = _moving_dtype(mode)
    np_dt = {
        mybir.dt.float32: np.float32,
        mybir.dt.bfloat16: None,  # filled below (ml_dtypes)
        mybir.dt.float16: np.float16,
    }[mdt]
    if np_dt is None:
        import ml_dtypes

        np_dt = ml_dtypes.bfloat16

    V = np.asarray(V, dtype=np.float32)
    W1 = np.asarray(W1, dtype=np.float32)
    b1 = np.asarray(b1, dtype=np.float32)
    W2 = np.asarray(W2, dtype=np.float32)
    b2 = np.asarray(b2, dtype=np.float32)

    # Prepack W1 into the SBUF tile layout [P, KC*HIDDEN]:
    # element (c*P + p, h) of W1 lands at [p, c*HIDDEN + h].
    w1p = np.ascontiguousarray(
        W1.astype(np_dt).reshape(KC, P, HIDDEN).transpose(1, 0, 2).reshape(P, KC * HIDDEN)
    )
    common = {
        "W1P": w1p,
        "B1": np.ascontiguousarray(b1.reshape(HIDDEN, 1)),
        "W2F": np.ascontiguousarray(W2.astype(np_dt)),
        "B2": np.ascontiguousarray(b2.reshape(OUT_DIM, 1)),
    }

    in_maps = []
    for c in range(NCORES):
        shard = V[c * R : (c + 1) * R]  # [R, IN_DIM]
        m = {"VTH": np.ascontiguousarray(shard.T.astype(np_dt))}
        m.update(common)
        in_maps.append(m)

    nc = build_nc(mode, R)
    nc.finalize()
    res = run_bass_kernel_spmd(nc, in_maps, list(range(NCORES)))
    _last_results = res

    out = np.concatenate(
        [np.asarray(r["OUT"]).T for r in res.results], axis=0
    ).astype(np.float32)
    return out
